# revision 10
# baseline (speedup 1.0000x reference)
"""NT-Xent (SimCLR) contrastive loss on 8 Trainium2 NeuronCores.

Data-parallel, collective-free, single-pass design:
  - Host casts embeddings to bf16, stacks E=[emb_i;emb_j] [8192,256] and
    replicates it to every core; core c additionally gets its own 512-row
    slices of emb_i/emb_j (so the kernel is core-id independent).
  - Per core, one flat pipeline over 8 batches of 8 row-chunks (one row per
    partition, row = c*128 + p):
      * HWDGE load -> SBUF bf16 (no f32 traffic, no DRAM round-trip)
      * square (GPSIMD/DVE split) + DVE d-half add + half-width row-reduce
        -> n2; ACT Ln + Exp(-0.5) -> 1/|e| (both in the natural_log_exp
        table: no table reloads)
      * DVE tensor_scalar_mul -> unit-norm z chunk (bf16, 2x DVE mode)
      * one SBUF->SBUF xbar DMA-transpose per batch builds ztb[p, c, r, q]
        = z[c*128+q, r*128+p]: chunk-major z^T whose 8-chunk dst region is
        contiguous, so 9 transpose instructions cover all 8704 rows
      * 16 bf16 matmuls (K=2x128) per batch accumulate into a [128,2048]
        PSUM tile shared by batch pairs; ACT Exp(scale=2, accum_out) fuses
        exp and row-sum per 2048 columns. Self-logit exp(2|z|^2)=e^2 is
        subtracted via the Ln bias.
  - Positive pairs come from a DVE multiply + reduce on the own-row chunks;
    own z_i^T (zTo) is one more 4-chunk transpose, and is the stationary
    matmul operand.
  - Output: per-row loss terms [128,4] per core; host sums 4096 values.
"""

import sys

if "/opt/trn_rl_repo" not in sys.path:
    sys.path.insert(0, "/opt/trn_rl_repo")

import numpy as np
import ml_dtypes

import concourse.bass as bass
import concourse.mybir as mybir
import concourse.tile as tile
from concourse import bass_utils

N_CORES = 8
N = 4096          # pairs
D = 256           # embedding dim
ROWS_ALL = 2 * N                      # stacked rows
OUT_ROWS = N // N_CORES               # 512 loss rows per core
INV_T = 2.0                           # 1 / temperature
E2_SELF = float(np.float32(np.exp(np.float32(2.0))))

FP32 = mybir.dt.float32
BF16 = mybir.dt.bfloat16

AF = mybir.ActivationFunctionType
ALU = mybir.AluOpType

N_CHUNKS = ROWS_ALL // 128            # 64 row-chunks, one row per partition
N_BATCH = 8                           # pipeline batches of 8 chunks
CPB = N_CHUNKS // N_BATCH             # chunks per batch (8)
N_GPSIMD_SQ = 12                      # of 16 square groups, run this many on GPSIMD


def _split_oversized_waits(nc, max_waits=1):
    """Walrus accepts at most one sync-wait per instruction; hoist extras
    onto preceding single-wait drains on the same engine (streams are FIFO
    per engine, so semantics are preserved)."""
    for bb in nc.main_func.blocks:
        new_list = []
        for ins in bb.instructions:
            si = ins.sync_info
            if si is not None and si.on_wait and len(si.on_wait) > max_waits:
                waits = list(si.on_wait)
                extra, keep = waits[:-max_waits], waits[-max_waits:]
                for gi, w in enumerate(extra):
                    d = mybir.InstDrain(name=f"{ins.name}-wsplit{gi}", engine=ins.engine)
                    d.sync_info = mybir.SyncInfo(on_wait=[w], on_update=[])
                    new_list.append(d)
                ins.sync_info = mybir.SyncInfo(on_wait=list(keep), on_update=list(si.on_update))
            new_list.append(ins)
        bb.instructions = new_list
    return nc


def _build():
    nc = bass.Bass("TRN2", num_devices=N_CORES)
    e_full = nc.dram_tensor("e_full", [ROWS_ALL, D], BF16, kind="ExternalInput")
    e_own_i = nc.dram_tensor("e_own_i", [OUT_ROWS, D], BF16, kind="ExternalInput")
    e_own_j = nc.dram_tensor("e_own_j", [OUT_ROWS, D], BF16, kind="ExternalInput")
    pp_out = nc.dram_tensor("pp_out", [128, 4], FP32, kind="ExternalOutput")

    # one row per partition: row = c*128 + p
    full_v = e_full.ap().rearrange("(c p) d -> p c d", p=128)    # [128, 64, 256]
    own_i_v = e_own_i.ap().rearrange("(c p) d -> p c d", p=128)  # [128, 4, 256]
    own_j_v = e_own_j.ap().rearrange("(c p) d -> p c d", p=128)

    def half_reduce(nc, pool, n2_slice, sq):
        """n2_slice [128, 4] = rowsum of sq [128, 4, 256] via a 2x-mode
        d-half add followed by a half-width reduce."""
        dh = pool.tile([128, 4, D // 2], BF16, tag="dh")
        nc.vector.tensor_add(dh, sq[:, :, 0:D // 2], sq[:, :, D // 2:D])
        nc.vector.tensor_reduce(n2_slice, dh, axis=mybir.AxisListType.X,
                                op=ALU.add)

    with tile.TileContext(nc) as tc:
        with tc.tile_pool(name="persist", bufs=1) as persist, \
             tc.tile_pool(name="work", bufs=4) as work, \
             tc.tile_pool(name="sqp", bufs=3) as sqp, \
             tc.tile_pool(name="escp", bufs=3) as escp, \
             tc.tile_pool(name="psum", bufs=2, space="PSUM") as psum:

            neg_e2 = persist.tile([128, 1], FP32)
            nc.vector.memset(neg_e2, -E2_SELF)

            n2 = persist.tile([128, N_CHUNKS], FP32)
            inv = persist.tile([128, N_CHUNKS], FP32)
            rs = persist.tile([128, 4, 5], FP32)     # exp row-sums (m, group)
            pos2 = persist.tile([128, 4], FP32)      # z_i . z_j per own row
            n2o = persist.tile([128, 8], FP32)
            invo = persist.tile([128, 8], FP32)
            ppsb = persist.tile([128, 4], FP32)
            # transposed z: ztb[p, c, r, q] = z[row c*128+q, dim r*128+p]
            zTo = persist.tile([128, 4, 2, 128], BF16)        # own z_i^T
            ztb = persist.tile([128, N_CHUNKS, 2, 128], BF16)  # all z^T
            z = persist.tile([128, N_CHUNKS, D], BF16)
            owni = persist.tile([128, 4, D], BF16)
            ownj = persist.tile([128, 4, D], BF16)
            zio = persist.tile([128, 4, D], BF16)
            zjo = persist.tile([128, 4, D], BF16)
            ebf = [persist.tile([128, 4, D], BF16, tag=f"ebf{g}", name=f"ebf{g}")
                   for g in range(16)]

            # head of the SP ring: start streaming the big input first
            nc.sync.dma_start(ebf[0], full_v[:, 0:4, :])
            nc.sync.dma_start(ebf[1], full_v[:, 4:8, :])
            nc.sync.dma_start(owni, own_i_v)
            nc.sync.dma_start(ownj, own_j_v)
            nc.sync.dma_start(ebf[2], full_v[:, 8:12, :])
            nc.sync.dma_start(ebf[3], full_v[:, 12:16, :])

            def own_phase():
                for src, cols in ((owni, slice(0, 4)), (ownj, slice(4, 8))):
                    sqo = work.tile([128, 4, D], BF16, tag="sqo")
                    nc.vector.tensor_mul(sqo, src, src)
                    half_reduce(nc, work, n2o[:, cols], sqo)
                lno = work.tile([128, 8], FP32, tag="lno")
                nc.scalar.activation(lno, n2o, AF.Ln)
                nc.scalar.activation(invo, lno, AF.Exp, scale=-0.5)
                for c in range(4):
                    nc.vector.tensor_scalar_mul(zio[:, c, :], owni[:, c, :],
                                                invo[:, c:c + 1])
                    nc.vector.tensor_scalar_mul(zjo[:, c, :], ownj[:, c, :],
                                                invo[:, 4 + c:5 + c])
                pprod = work.tile([128, 4, D], BF16, tag="pprod")
                nc.vector.tensor_mul(pprod, zio, zjo)
                half_reduce(nc, work, pos2, pprod)
                nc.sync.dma_start_transpose(zTo, zio)

            def batch(b):
                """load prefetch + normalize + transpose for chunks
                [8b, 8b+8); batch-0 squares run on DVE for a fast fill."""
                for h in range(2):
                    g = 2 * b + h
                    if g + 4 < 16:
                        nc.sync.dma_start(ebf[g + 4],
                                          full_v[:, 4 * (g + 4):4 * (g + 5), :])
                    sq = sqp.tile([128, 4, D], BF16, tag="sq")
                    sq_eng = nc.vector if b == 0 else nc.gpsimd
                    sq_eng.tensor_mul(sq, ebf[g], ebf[g])
                    half_reduce(nc, work, n2[:, 4 * g:4 * g + 4], sq)
                c0 = CPB * b
                lng = work.tile([128, 8], FP32, tag="lng")
                nc.scalar.activation(lng, n2[:, c0:c0 + 8], AF.Ln)
                nc.scalar.activation(inv[:, c0:c0 + 8], lng, AF.Exp,
                                     scale=-0.5)
                for k in range(CPB):
                    c = c0 + k
                    nc.vector.tensor_scalar_mul(
                        z[:, c, :], ebf[2 * b + k // 4][:, k % 4, :],
                        inv[:, c:c + 1])
                nc.sync.dma_start_transpose(ztb[:, c0:c0 + CPB, :, :],
                                            z[:, c0:c0 + CPB, :])

            def mm_group(G, cg0, cg1):
                """matmuls + fused exp row-sum over z columns [cg0*128,
                cg1*128) — a 1024- or 2048-wide PSUM window."""
                w = (cg1 - cg0) * 128
                for m in range(4):
                    S = psum.tile([128, 2048], FP32, tag="S")
                    for cc in range((cg1 - cg0) // 4):
                        for r in range(2):
                            nc.tensor.matmul(
                                S[:, cc * 512:(cc + 1) * 512],
                                zTo[:, m, r, :],
                                ztb[:, cg0 + 4 * cc:cg0 + 4 * cc + 4, r, :],
                                start=(r == 0), stop=(r == 1))
                    esc = escp.tile([128, 2048], BF16, tag="esc")
                    nc.scalar.activation(esc[:, 0:w], S[:, 0:w], AF.Exp,
                                         scale=INV_T,
                                         accum_out=rs[:, m, G:G + 1])

            # ---------------- main pipeline: 8 batches of 8 chunks, -------
            # ---------------- 5 matmul/exp column groups ------------------
            batch(0)
            own_phase()
            mm_group(0, 0, 8)
            batch(1)
            batch(2)
            mm_group(1, 8, 24)
            batch(3)
            batch(4)
            mm_group(2, 24, 40)
            batch(5)
            batch(6)
            mm_group(3, 40, 56)
            batch(7)
            mm_group(4, 56, 64)

            # ---------------- finish: log-denominator minus positives -------
            rtot = work.tile([128, 4], FP32, tag="rtot")
            nc.vector.tensor_reduce(rtot, rs, axis=mybir.AxisListType.X,
                                    op=ALU.add)
            logden = work.tile([128, 4], FP32, tag="logden")
            nc.scalar.activation(logden, rtot, AF.Ln, bias=neg_e2[:, 0:1])
            nc.vector.scalar_tensor_tensor(
                out=ppsb, in0=pos2, scalar=-INV_T,
                in1=logden, op0=ALU.mult, op1=ALU.add)

            nc.sync.dma_start(pp_out.ap(), ppsb)

    _split_oversized_waits(nc)
    return nc


_NC_CACHE = None


def _get_nc():
    global _NC_CACHE
    if _NC_CACHE is None:
        _NC_CACHE = _build()
    return _NC_CACHE


def _make_in_maps(emb_i: np.ndarray, emb_j: np.ndarray):
    emb_i = np.asarray(emb_i, dtype=np.float32)
    emb_j = np.asarray(emb_j, dtype=np.float32)
    bf = ml_dtypes.bfloat16
    e_full = np.ascontiguousarray(
        np.concatenate([emb_i, emb_j], axis=0).astype(bf))
    ei = np.ascontiguousarray(emb_i.astype(bf))
    ej = np.ascontiguousarray(emb_j.astype(bf))
    in_maps = []
    for c in range(N_CORES):
        sl = slice(c * OUT_ROWS, (c + 1) * OUT_ROWS)
        in_maps.append({
            "e_full": e_full,
            "e_own_i": ei[sl],
            "e_own_j": ej[sl],
        })
    return in_maps


def kernel(emb_i: np.ndarray, emb_j: np.ndarray) -> np.ndarray:
    nc = _get_nc()
    in_maps = _make_in_maps(emb_i, emb_j)
    res = bass_utils.run_bass_kernel_spmd(nc, in_maps, core_ids=list(range(N_CORES)))
    total = 0.0
    for c in range(N_CORES):
        total += res.results[c]["pp_out"].astype(np.float64).sum()
    return np.float32(total / N)
